# revision 8
# baseline (speedup 1.0000x reference)
"""Multi-head self-attention (B=2, S=2048, E=1024, H=16, causal) on 8 trn2 cores.

Sharding: core c handles batch b = c // 4 and heads [4*(c%4), 4*(c%4)+4).
Each core computes its 4 heads' attention output and a partial output
projection (row-sharded Wout); the host sums the 4 partials per batch and
adds bout.

On-core layout (all matmul operands bf16, fp32 PSUM accumulation):
  qt    [E=1024, S=2048]  Q[b]^T           (DMA in, bf16)
  qT/kT [256, S]  per-head-dim-transposed projections, scale 1/8 folded in q
  v     [S, 4, 65] natural layout, per-head 65th column = ones (for softmax sum)
  scoresT tiles [128 k, 512 q]  (lhsT = kT chunk [64,128], rhs = qT [64,512])
  p = exp(scoresT)  bf16, causal-zeroed via gpsimd affine_select/memset
  av psum [65, 1024]: row 64 accumulates softmax denominator l
  out^T = (av rows 0..63) * broadcast(1/l)  -> bf16, feeds output projection
"""

import os
from contextlib import ExitStack

import ml_dtypes
import numpy as np

import concourse.bass as bass
import concourse.mybir as mybir
import concourse.tile as tile
from concourse import bacc
from concourse.bass_utils import run_bass_kernel_spmd

f32 = mybir.dt.float32
bf16 = mybir.dt.bfloat16
bfnp = ml_dtypes.bfloat16

S = 2048
E = 1024
HC = 4  # heads per core
D = 64
C = HC * D  # 256 per-core head dims
NE = E // 128  # 8 contraction chunks

Exp = mybir.ActivationFunctionType.Exp
Ln = mybir.ActivationFunctionType.Ln
Ident = mybir.ActivationFunctionType.Identity


def _build_kernel(tc, qt, wq, wk, wv, wo, bq, bk, bv, y):
    nc = tc.nc
    rrow = nc.dram_tensor("rrow", [8, 1024], f32).ap()
    with ExitStack() as ctx:
        const = ctx.enter_context(tc.tile_pool(name="const", bufs=1))
        qt_sb = const.tile([128, NE, S], bf16)
        wq_sb = const.tile([128, NE, C], bf16)
        wk_sb = const.tile([128, NE, C], bf16)
        wv_sb = const.tile([128, NE, C], bf16)
        wo_sb = const.tile([128, 2, E], bf16)
        bq_sb = const.tile([128, 2], f32)
        bk_sb = const.tile([128, 2], f32)
        bv_sb = const.tile([1, C], bf16)
        ones_sb = const.tile([1, 128], bf16)
        qT_sb = const.tile([128, 2, S], bf16)
        kT_sb = const.tile([128, 2, S], bf16)
        v_sb = [
            const.tile([128, HC, D + 1], bf16, tag=f"v{si}", name=f"v_sb{si}")
            for si in range(16)
        ]
        out_sb = const.tile([128, 2, S], bf16)

        # --- loads ---
        qt_r = qt.rearrange("(i p) s -> p i s", p=128)
        for i in range(NE):
            nc.sync.dma_start(qt_sb[:, i, :], qt_r[:, i, :])
        nc.sync.dma_start(wq_sb[:], wq.rearrange("(i p) c -> p i c", p=128))
        nc.sync.dma_start(wk_sb[:], wk.rearrange("(i p) c -> p i c", p=128))
        nc.sync.dma_start(wv_sb[:], wv.rearrange("(i p) c -> p i c", p=128))
        nc.sync.dma_start(wo_sb[:], wo.rearrange("(m p) e -> p m e", p=128))
        nc.sync.dma_start(bq_sb[:], bq[:])
        nc.sync.dma_start(bk_sb[:], bk[:])
        nc.sync.dma_start(bv_sb[:], bv[:])
        nc.vector.memset(ones_sb[:], 1.0)
        for si in range(16):
            nc.gpsimd.memset(v_sb[si][:, :, D : D + 1], 1.0)

        # --- qkv projections ---
        with tc.tile_pool(name="pqk", bufs=3, space="PSUM") as pqk, tc.tile_pool(
            name="pv", bufs=3, space="PSUM"
        ) as pv:
            for m in range(2):
                for g in range(4):
                    for wsb, dst, bsb, scale in (
                        (wq_sb, qT_sb, bq_sb, 0.125),
                        (wk_sb, kT_sb, bk_sb, 1.0),
                    ):
                        ps = pqk.tile([128, 512], f32, tag="pqk")
                        for i in range(NE):
                            nc.tensor.matmul(
                                ps[:],
                                lhsT=wsb[:, i, 128 * m : 128 * m + 128],
                                rhs=qt_sb[:, i, 512 * g : 512 * g + 512],
                                start=(i == 0),
                                stop=(i == NE - 1),
                            )
                        nc.scalar.activation(
                            dst[:, m, 512 * g : 512 * g + 512],
                            ps[:],
                            Ident,
                            bias=bsb[:, m : m + 1],
                            scale=scale,
                        )
            for si in range(16):
                ps = pv.tile([128, C], f32, tag="pv")
                for i in range(NE):
                    nc.tensor.matmul(
                        ps[:],
                        lhsT=qt_sb[:, i, 128 * si : 128 * si + 128],
                        rhs=wv_sb[:, i, :],
                        start=(i == 0),
                        stop=False,
                    )
                nc.tensor.matmul(
                    ps[:],
                    lhsT=ones_sb[:, 0:128],
                    rhs=bv_sb[:],
                    start=False,
                    stop=True,
                )
                nc.vector.tensor_copy(
                    v_sb[si][:, :, 0:D],
                    ps[:].rearrange("p (h d) -> p h d", h=HC),
                )

        # --- attention (head pairs share the PE array via row groups 0/64) ---
        with tc.tile_pool(name="psc", bufs=2, space="PSUM") as psc, tc.tile_pool(
            name="pav", bufs=1, space="PSUM"
        ) as pav, tc.tile_pool(name="ppool", bufs=6) as ppool, tc.tile_pool(
            name="rl", bufs=2
        ) as rl:
            for pr in range(2):  # head pair = c-chunk index
                for Hh in range(2):  # q halves of 1024
                    av = [
                        pav.tile(
                            [D + 1, 1024], f32, tag=f"av{j}", name=f"av{pr}_{Hh}_{j}"
                        )
                        for j in range(2)
                    ]
                    for g in (2 * Hh, 2 * Hh + 1):
                        gc = 512 * (g - 2 * Hh)
                        for kc in range(4 * g + 4):
                            pss = []
                            for j in range(2):
                                b0 = 64 * j
                                psj = psc.tile([128, 512], f32, tag=f"sc{j}")
                                nc.tensor.matmul(
                                    psj[:],
                                    lhsT=kT_sb[
                                        b0 : b0 + 64, pr, 128 * kc : 128 * kc + 128
                                    ],
                                    rhs=qT_sb[
                                        b0 : b0 + 64, pr, 512 * g : 512 * g + 512
                                    ],
                                    start=True,
                                    stop=True,
                                )
                                pss.append(psj)
                            for j in range(2):
                                pt = ppool.tile([128, 512], bf16, tag="p")
                                md = kc - 4 * g
                                if md < 0:
                                    nc.scalar.activation(pt[:], pss[j][:], Exp)
                                else:
                                    if md > 0:
                                        nc.gpsimd.memset(pt[:, 0 : 128 * md], 0.0)
                                    nc.scalar.activation(
                                        pt[:, 128 * md : 512],
                                        pss[j][:, 128 * md : 512],
                                        Exp,
                                    )
                                    blk = pt[:, 128 * md : 128 * md + 128]
                                    nc.gpsimd.affine_select(
                                        out=blk,
                                        in_=blk,
                                        pattern=[[1, 128]],
                                        compare_op=mybir.AluOpType.is_ge,
                                        fill=0.0,
                                        base=0,
                                        channel_multiplier=-1,
                                    )
                                nc.tensor.matmul(
                                    av[j][:, gc : gc + 512],
                                    lhsT=v_sb[kc][:, 2 * pr + j, :],
                                    rhs=pt[:],
                                    start=(kc == 0),
                                    stop=(kc == 4 * g + 3),
                                )
                    for j in range(2):
                        # l (softmax denom) sits on psum partition 64; keep all
                        # intermediates at base partition 64 to avoid any
                        # cross-partition engine ops, then DMA-broadcast 1/l.
                        l_sb = rl.tile([D + 1, 1024], f32, tag="l")
                        nc.vector.tensor_copy(l_sb[D : D + 1, :], av[j][D : D + 1, :])
                        nc.scalar.activation(l_sb[D : D + 1, :], l_sb[D : D + 1, :], Ln)
                        nc.scalar.activation(
                            l_sb[D : D + 1, :], l_sb[D : D + 1, :], Exp, scale=-1.0
                        )
                        rb = rl.tile([64, 1024], f32, tag="rb")
                        ridx = 4 * pr + 2 * Hh + j
                        nc.sync.dma_start(rrow[ridx : ridx + 1, :], l_sb[D : D + 1, :])
                        rr = rrow[ridx, :]
                        nc.sync.dma_start(
                            rb[:],
                            bass.AP(
                                tensor=rr.tensor,
                                offset=rr.offset,
                                ap=[[0, 64], [1, 1024]],
                            ),
                        )
                        nc.vector.tensor_mul(
                            out_sb[
                                64 * j : 64 * j + 64, pr, 1024 * Hh : 1024 * Hh + 1024
                            ],
                            av[j][0:D, :],
                            rb[:],
                        )

        # --- output projection (partial: this core's 256 contraction rows) ---
        with tc.tile_pool(name="py", bufs=4, space="PSUM") as py, tc.tile_pool(
            name="ysb", bufs=4
        ) as ysb:
            for t in range(16):
                for e in range(2):
                    ps = py.tile([128, 512], f32, tag="py")
                    for m in range(2):
                        nc.tensor.matmul(
                            ps[:],
                            lhsT=out_sb[:, m, 128 * t : 128 * t + 128],
                            rhs=wo_sb[:, m, 512 * e : 512 * e + 512],
                            start=(m == 0),
                            stop=(m == 1),
                        )
                    yt = ysb.tile([128, 512], f32, tag="yt")
                    if e == 0:
                        nc.vector.tensor_copy(yt[:], ps[:])
                    else:
                        nc.scalar.copy(yt[:], ps[:])
                    nc.sync.dma_start(
                        y[128 * t : 128 * t + 128, 512 * e : 512 * e + 512], yt[:]
                    )


_NC = None


def build_nc():
    global _NC
    if _NC is not None:
        return _NC
    nc = bacc.Bacc("TRN2", target_bir_lowering=False, debug=False, num_devices=8)
    qt = nc.dram_tensor("qt", [E, S], bf16, kind="ExternalInput").ap()
    wq = nc.dram_tensor("wq", [E, C], bf16, kind="ExternalInput").ap()
    wk = nc.dram_tensor("wk", [E, C], bf16, kind="ExternalInput").ap()
    wv = nc.dram_tensor("wv", [E, C], bf16, kind="ExternalInput").ap()
    wo = nc.dram_tensor("wo", [C, E], bf16, kind="ExternalInput").ap()
    bq = nc.dram_tensor("bq", [128, 2], f32, kind="ExternalInput").ap()
    bk = nc.dram_tensor("bk", [128, 2], f32, kind="ExternalInput").ap()
    bv = nc.dram_tensor("bv", [1, C], bf16, kind="ExternalInput").ap()
    y = nc.dram_tensor("y", [S, E], f32, kind="ExternalOutput").ap()
    with tile.TileContext(nc) as tc:
        _build_kernel(tc, qt, wq, wk, wv, wo, bq, bk, bv, y)
    nc.compile()
    _NC = nc
    return nc


def make_in_maps(Q, Wqkv, bqkv):
    """Per-core input dicts (8 cores: batch-major, then head-group)."""
    in_maps = []
    for c in range(8):
        b, hq = c // 4, c % 4
        cs = C * hq
        qt_np = np.ascontiguousarray(Q[b].T).astype(bfnp)
        wq_np = np.ascontiguousarray(Wqkv[:, cs : cs + C]).astype(bfnp)
        wk_np = np.ascontiguousarray(Wqkv[:, E + cs : E + cs + C]).astype(bfnp)
        wv_np = np.ascontiguousarray(Wqkv[:, 2 * E + cs : 2 * E + cs + C]).astype(bfnp)
        bq_np = np.ascontiguousarray(
            (bqkv[cs : cs + C].astype(np.float32) * 0.125).reshape(2, 128).T
        )
        bk_np = np.ascontiguousarray(
            bqkv[E + cs : E + cs + C].astype(np.float32).reshape(2, 128).T
        )
        bv_np = bqkv[2 * E + cs : 2 * E + cs + C].reshape(1, C).astype(bfnp)
        in_maps.append(
            {
                "qt": qt_np,
                "wq": wq_np,
                "wk": wk_np,
                "wv": wv_np,
                "bq": bq_np,
                "bk": bk_np,
                "bv": bv_np,
            }
        )
    return in_maps


def kernel(Q, Wqkv, bqkv, Wout, bout, _trace=False, _trace_kwargs=None):
    Q = np.asarray(Q, dtype=np.float32)
    Wqkv = np.asarray(Wqkv, dtype=np.float32)
    bqkv = np.asarray(bqkv, dtype=np.float32)
    Wout = np.asarray(Wout, dtype=np.float32)
    bout = np.asarray(bout, dtype=np.float32)

    nc = build_nc()
    in_maps = make_in_maps(Q, Wqkv, bqkv)
    for c in range(8):
        hq = c % 4
        cs = C * hq
        in_maps[c]["wo"] = np.ascontiguousarray(Wout[cs : cs + C, :]).astype(bfnp)

    kwargs = {}
    if _trace:
        kwargs = dict(trace=True, trace_cores=list(range(8)))
        if _trace_kwargs:
            kwargs.update(_trace_kwargs)
    res = run_bass_kernel_spmd(nc, in_maps, core_ids=list(range(8)), **kwargs)

    out = np.zeros((2, S, E), dtype=np.float32)
    for c in range(8):
        out[c // 4] += np.asarray(res.results[c]["y"], dtype=np.float32)
    out += bout.astype(np.float32)[None, None, :]
    if _trace:
        kernel._last_results = res
    return out


# revision 10
# speedup vs baseline: 1.1254x; 1.1254x over previous
"""Multi-head self-attention (B=2, S=2048, E=1024, H=16, causal) on 8 trn2 cores.

Sharding: core c handles batch b = c // 4 and heads [4*(c%4), 4*(c%4)+4).
Each core computes its 4 heads' attention output and a partial output
projection (row-sharded Wout); the host sums the 4 partials per batch and
adds bout.

On-core layout (all matmul operands bf16, fp32 PSUM accumulation):
  qt    [E=1024, S=2048]  Q[b]^T           (DMA in, bf16)
  qT/kT [256, S]  per-head-dim-transposed projections, scale 1/8 folded in q
  v     [S, 4, 65] natural layout, per-head 65th column = ones (for softmax sum)
  scoresT tiles [128 k, 512 q]  (lhsT = kT chunk [64,128], rhs = qT [64,512])
  p = exp(scoresT)  bf16, causal-zeroed via gpsimd affine_select/memset
  av psum [65, 1024]: row 64 accumulates softmax denominator l
  out^T = (av rows 0..63) * broadcast(1/l)  -> bf16, feeds output projection
"""

import os
from contextlib import ExitStack

import ml_dtypes
import numpy as np

import concourse.bass as bass
import concourse.mybir as mybir
import concourse.tile as tile
from concourse import bacc
from concourse.bass_utils import run_bass_kernel_spmd

f32 = mybir.dt.float32
bf16 = mybir.dt.bfloat16
bfnp = ml_dtypes.bfloat16

S = 2048
E = 1024
HC = 4  # heads per core
D = 64
C = HC * D  # 256 per-core head dims
NE = E // 128  # 8 contraction chunks

Exp = mybir.ActivationFunctionType.Exp
Ln = mybir.ActivationFunctionType.Ln
Ident = mybir.ActivationFunctionType.Identity


def _build_kernel(tc, qt, wq, wk, wv, wo, bq, bk, bv, y):
    nc = tc.nc
    rrow = nc.dram_tensor("rrow", [8, 1024], f32).ap()
    with ExitStack() as ctx:
        const = ctx.enter_context(tc.tile_pool(name="const", bufs=1))
        qt_sb = const.tile([128, NE, S], bf16)
        wq_sb = const.tile([128, NE, C], bf16)
        wk_sb = const.tile([128, NE, C], bf16)
        wv_sb = const.tile([128, NE, C], bf16)
        wo_sb = const.tile([128, 2, E], bf16)
        bq_sb = const.tile([128, 2], f32)
        bk_sb = const.tile([128, 2], f32)
        bv_sb = const.tile([1, C], bf16)
        ones_sb = const.tile([1, 128], bf16)
        qT_sb = const.tile([128, 2, S], bf16)
        kT_sb = const.tile([128, 2, S], bf16)
        v_sb = [
            const.tile([128, HC, D + 1], bf16, tag=f"v{si}", name=f"v_sb{si}")
            for si in range(16)
        ]
        out_sb = const.tile([128, 2, S], bf16)

        # --- loads ---
        qt_r = qt.rearrange("(i p) s -> p i s", p=128)
        for i in range(NE):
            nc.sync.dma_start(qt_sb[:, i, :], qt_r[:, i, :])
        nc.sync.dma_start(wq_sb[:], wq.rearrange("(i p) c -> p i c", p=128))
        nc.sync.dma_start(wk_sb[:], wk.rearrange("(i p) c -> p i c", p=128))
        nc.sync.dma_start(wv_sb[:], wv.rearrange("(i p) c -> p i c", p=128))
        nc.sync.dma_start(wo_sb[:], wo.rearrange("(m p) e -> p m e", p=128))
        nc.sync.dma_start(bq_sb[:], bq[:])
        nc.sync.dma_start(bk_sb[:], bk[:])
        nc.sync.dma_start(bv_sb[:], bv[:])
        nc.vector.memset(ones_sb[:], 1.0)
        for si in range(16):
            nc.gpsimd.memset(v_sb[si][:, :, D : D + 1], 1.0)

        # --- qkv projections ---
        with tc.tile_pool(name="pqk", bufs=3, space="PSUM") as pqk, tc.tile_pool(
            name="pv", bufs=3, space="PSUM"
        ) as pv:
            for m in range(2):
                for g in range(4):
                    for wsb, dst, bsb, scale in (
                        (wq_sb, qT_sb, bq_sb, 0.125),
                        (wk_sb, kT_sb, bk_sb, 1.0),
                    ):
                        ps = pqk.tile([128, 512], f32, tag="pqk")
                        for i in range(NE):
                            nc.tensor.matmul(
                                ps[:],
                                lhsT=wsb[:, i, 128 * m : 128 * m + 128],
                                rhs=qt_sb[:, i, 512 * g : 512 * g + 512],
                                start=(i == 0),
                                stop=(i == NE - 1),
                            )
                        if scale == 1.0:
                            nc.vector.tensor_scalar_add(
                                dst[:, m, 512 * g : 512 * g + 512],
                                ps[:],
                                bsb[:, m : m + 1],
                            )
                        else:
                            nc.vector.tensor_scalar(
                                dst[:, m, 512 * g : 512 * g + 512],
                                ps[:],
                                scalar1=bsb[:, m : m + 1],
                                scalar2=scale,
                                op0=mybir.AluOpType.add,
                                op1=mybir.AluOpType.mult,
                            )
            for si in range(16):
                ps = pv.tile([128, C], f32, tag="pv")
                for i in range(NE):
                    nc.tensor.matmul(
                        ps[:],
                        lhsT=qt_sb[:, i, 128 * si : 128 * si + 128],
                        rhs=wv_sb[:, i, :],
                        start=(i == 0),
                        stop=False,
                    )
                nc.tensor.matmul(
                    ps[:],
                    lhsT=ones_sb[:, 0:128],
                    rhs=bv_sb[:],
                    start=False,
                    stop=True,
                )
                nc.vector.tensor_copy(
                    v_sb[si][:, :, 0:D],
                    ps[:].rearrange("p (h d) -> p h d", h=HC),
                )

        # --- attention (head pairs share the PE array via row groups 0/64) ---
        with tc.tile_pool(name="psc", bufs=1, space="PSUM") as psc, tc.tile_pool(
            name="pav", bufs=1, space="PSUM"
        ) as pav, tc.tile_pool(name="ppool", bufs=4) as ppool, tc.tile_pool(
            name="rl", bufs=2
        ) as rl:
            for pr in range(2):  # head pair = c-chunk index
                for Hh in range(2):  # q halves of 1024
                    q0 = 1024 * Hh
                    av = [
                        pav.tile(
                            [D + 1, 1024], f32, tag=f"av{j}", name=f"av{pr}_{Hh}_{j}"
                        )
                        for j in range(2)
                    ]
                    for kc in range(8 * Hh + 8):
                        md = kc - 8 * Hh
                        for j in range(2):
                            b0 = 64 * j
                            psj = psc.tile(
                                [128, 1024], f32, tag=f"sc{j}", name=f"sc{j}_{kc}"
                            )
                            for g2 in range(2):
                                if md >= 4 and g2 == 0:
                                    continue
                                nc.tensor.matmul(
                                    psj[:, 512 * g2 : 512 * g2 + 512],
                                    lhsT=kT_sb[
                                        b0 : b0 + 64, pr, 128 * kc : 128 * kc + 128
                                    ],
                                    rhs=qT_sb[
                                        b0 : b0 + 64,
                                        pr,
                                        q0 + 512 * g2 : q0 + 512 * g2 + 512,
                                    ],
                                    start=True,
                                    stop=True,
                                )
                            pt = ppool.tile([128, 1024], bf16, tag="p")
                            e0 = max(0, 128 * md)
                            nc.scalar.activation(pt[:, e0:1024], psj[:, e0:1024], Exp)
                            if 0 <= md <= 7:
                                zs = 0 if md < 4 else 512
                                if 128 * md > zs:
                                    nc.gpsimd.memset(pt[:, zs : 128 * md], 0.0)
                                blk = pt[:, 128 * md : 128 * md + 128]
                                nc.gpsimd.affine_select(
                                    out=blk,
                                    in_=blk,
                                    pattern=[[1, 128]],
                                    compare_op=mybir.AluOpType.is_ge,
                                    fill=0.0,
                                    base=0,
                                    channel_multiplier=-1,
                                )
                            for g2 in range(2):
                                if md >= 4 and g2 == 0:
                                    continue
                                nc.tensor.matmul(
                                    av[j][:, 512 * g2 : 512 * g2 + 512],
                                    lhsT=v_sb[kc][:, 2 * pr + j, :],
                                    rhs=pt[:, 512 * g2 : 512 * g2 + 512],
                                    start=(kc == 0),
                                    stop=(kc == (8 * Hh + 3 if g2 == 0 else 8 * Hh + 7)),
                                )
                    for j in range(2):
                        # softmax denom l on psum partition 64 -> 1/l -> broadcast
                        l_sb = rl.tile([D + 1, 1024], f32, tag="l")
                        nc.vector.tensor_copy(l_sb[D : D + 1, :], av[j][D : D + 1, :])
                        ltall = rl.tile([128, 8], f32, tag="ltall")
                        l_row = l_sb[D : D + 1, :]
                        nc.sync.dma_start(
                            ltall[:],
                            bass.AP(
                                tensor=l_row.tensor,
                                offset=l_row.offset,
                                ap=[list(l_row.ap[0]), [8, 128], [1, 8]],
                            ),
                        )
                        nc.vector.reciprocal(ltall[:], ltall[:])
                        ridx = 4 * pr + 2 * Hh + j
                        nc.sync.dma_start(
                            rrow[ridx, :].rearrange("(p c) -> p c", p=128), ltall[:]
                        )
                        rb = rl.tile([64, 1024], f32, tag="rb")
                        rr = rrow[ridx, :]
                        nc.sync.dma_start(
                            rb[:],
                            bass.AP(
                                tensor=rr.tensor,
                                offset=rr.offset,
                                ap=[[0, 64], [1, 1024]],
                            ),
                        )
                        nc.vector.tensor_mul(
                            out_sb[
                                64 * j : 64 * j + 64, pr, q0 : q0 + 1024
                            ],
                            av[j][0:D, :],
                            rb[:],
                        )

        # --- output projection (partial: this core's 256 contraction rows) ---
        with tc.tile_pool(name="py", bufs=4, space="PSUM") as py, tc.tile_pool(
            name="ysb", bufs=4
        ) as ysb:
            for t in range(16):
                for e in range(2):
                    ps = py.tile([128, 512], f32, tag="py")
                    for m in range(2):
                        nc.tensor.matmul(
                            ps[:],
                            lhsT=out_sb[:, m, 128 * t : 128 * t + 128],
                            rhs=wo_sb[:, m, 512 * e : 512 * e + 512],
                            start=(m == 0),
                            stop=(m == 1),
                        )
                    yt = ysb.tile([128, 512], f32, tag="yt")
                    if e == 0:
                        nc.vector.tensor_copy(yt[:], ps[:])
                    else:
                        nc.scalar.copy(yt[:], ps[:])
                    nc.sync.dma_start(
                        y[128 * t : 128 * t + 128, 512 * e : 512 * e + 512], yt[:]
                    )


_NC = None


def build_nc():
    global _NC
    if _NC is not None:
        return _NC
    nc = bacc.Bacc("TRN2", target_bir_lowering=False, debug=False, num_devices=8)
    qt = nc.dram_tensor("qt", [E, S], bf16, kind="ExternalInput").ap()
    wq = nc.dram_tensor("wq", [E, C], bf16, kind="ExternalInput").ap()
    wk = nc.dram_tensor("wk", [E, C], bf16, kind="ExternalInput").ap()
    wv = nc.dram_tensor("wv", [E, C], bf16, kind="ExternalInput").ap()
    wo = nc.dram_tensor("wo", [C, E], bf16, kind="ExternalInput").ap()
    bq = nc.dram_tensor("bq", [128, 2], f32, kind="ExternalInput").ap()
    bk = nc.dram_tensor("bk", [128, 2], f32, kind="ExternalInput").ap()
    bv = nc.dram_tensor("bv", [1, C], bf16, kind="ExternalInput").ap()
    y = nc.dram_tensor("y", [S, E], f32, kind="ExternalOutput").ap()
    with tile.TileContext(nc) as tc:
        _build_kernel(tc, qt, wq, wk, wv, wo, bq, bk, bv, y)
    nc.compile()
    _NC = nc
    return nc


def make_in_maps(Q, Wqkv, bqkv):
    """Per-core input dicts (8 cores: batch-major, then head-group)."""
    in_maps = []
    for c in range(8):
        b, hq = c // 4, c % 4
        cs = C * hq
        qt_np = np.ascontiguousarray(Q[b].T).astype(bfnp)
        wq_np = np.ascontiguousarray(Wqkv[:, cs : cs + C]).astype(bfnp)
        wk_np = np.ascontiguousarray(Wqkv[:, E + cs : E + cs + C]).astype(bfnp)
        wv_np = np.ascontiguousarray(Wqkv[:, 2 * E + cs : 2 * E + cs + C]).astype(bfnp)
        bq_np = np.ascontiguousarray(
            bqkv[cs : cs + C].astype(np.float32).reshape(2, 128).T
        )
        bk_np = np.ascontiguousarray(
            bqkv[E + cs : E + cs + C].astype(np.float32).reshape(2, 128).T
        )
        bv_np = bqkv[2 * E + cs : 2 * E + cs + C].reshape(1, C).astype(bfnp)
        in_maps.append(
            {
                "qt": qt_np,
                "wq": wq_np,
                "wk": wk_np,
                "wv": wv_np,
                "bq": bq_np,
                "bk": bk_np,
                "bv": bv_np,
            }
        )
    return in_maps


def kernel(Q, Wqkv, bqkv, Wout, bout, _trace=False, _trace_kwargs=None):
    Q = np.asarray(Q, dtype=np.float32)
    Wqkv = np.asarray(Wqkv, dtype=np.float32)
    bqkv = np.asarray(bqkv, dtype=np.float32)
    Wout = np.asarray(Wout, dtype=np.float32)
    bout = np.asarray(bout, dtype=np.float32)

    nc = build_nc()
    in_maps = make_in_maps(Q, Wqkv, bqkv)
    for c in range(8):
        hq = c % 4
        cs = C * hq
        in_maps[c]["wo"] = np.ascontiguousarray(Wout[cs : cs + C, :]).astype(bfnp)

    kwargs = {}
    if _trace:
        kwargs = dict(trace=True, trace_cores=list(range(8)))
        if _trace_kwargs:
            kwargs.update(_trace_kwargs)
    res = run_bass_kernel_spmd(nc, in_maps, core_ids=list(range(8)), **kwargs)

    out = np.zeros((2, S, E), dtype=np.float32)
    for c in range(8):
        out[c // 4] += np.asarray(res.results[c]["y"], dtype=np.float32)
    out += bout.astype(np.float32)[None, None, :]
    if _trace:
        kernel._last_results = res
    return out


# revision 11
# speedup vs baseline: 1.1291x; 1.0033x over previous
"""Multi-head self-attention (B=2, S=2048, E=1024, H=16, causal) on 8 trn2 cores.

Sharding: core c handles batch b = c // 4 and heads [4*(c%4), 4*(c%4)+4).
Each core computes its 4 heads' attention output and a partial output
projection (row-sharded Wout); the host sums the 4 partials per batch and
adds bout.

On-core layout (all matmul operands bf16, fp32 PSUM accumulation):
  qt    [E=1024, S=2048]  Q[b]^T           (DMA in, bf16)
  qT/kT [256, S]  per-head-dim-transposed projections, scale 1/8 folded in q
  v     [S, 4, 65] natural layout, per-head 65th column = ones (for softmax sum)
  scoresT tiles [128 k, 512 q]  (lhsT = kT chunk [64,128], rhs = qT [64,512])
  p = exp(scoresT)  bf16, causal-zeroed via gpsimd affine_select/memset
  av psum [65, 1024]: row 64 accumulates softmax denominator l
  out^T = (av rows 0..63) * broadcast(1/l)  -> bf16, feeds output projection
"""

import os
from contextlib import ExitStack

import ml_dtypes
import numpy as np

import concourse.bass as bass
import concourse.mybir as mybir
import concourse.tile as tile
from concourse import bacc
from concourse.bass_utils import run_bass_kernel_spmd

f32 = mybir.dt.float32
bf16 = mybir.dt.bfloat16
bfnp = ml_dtypes.bfloat16

S = 2048
E = 1024
HC = 4  # heads per core
D = 64
C = HC * D  # 256 per-core head dims
NE = E // 128  # 8 contraction chunks

Exp = mybir.ActivationFunctionType.Exp
Ln = mybir.ActivationFunctionType.Ln
Ident = mybir.ActivationFunctionType.Identity


def _build_kernel(tc, qt, wq, wk, wv, wo, bq, bk, bv, y):
    nc = tc.nc
    rrow = nc.dram_tensor("rrow", [8, 1024], f32).ap()
    with ExitStack() as ctx:
        const = ctx.enter_context(tc.tile_pool(name="const", bufs=1))
        qt_sb = const.tile([128, NE, S], bf16)
        wq_sb = const.tile([128, NE, C], bf16)
        wk_sb = const.tile([128, NE, C], bf16)
        wv_sb = const.tile([128, NE, C], bf16)
        wo_sb = const.tile([128, 2, E], bf16)
        bq_sb = const.tile([128, 2], f32)
        bk_sb = const.tile([128, 2], f32)
        bv_sb = const.tile([1, C], bf16)
        ones_sb = const.tile([1, 128], bf16)
        qT_sb = const.tile([128, 2, S], bf16)
        kT_sb = const.tile([128, 2, S], bf16)
        v_sb = [
            const.tile([128, HC, D + 1], bf16, tag=f"v{si}", name=f"v_sb{si}")
            for si in range(16)
        ]
        out_sb = const.tile([128, 2, S], bf16)

        # --- loads ---
        qt_r = qt.rearrange("(i p) s -> p i s", p=128)
        for i in range(NE):
            nc.sync.dma_start(qt_sb[:, i, 0:1024], qt_r[:, i, 0:1024])
            nc.sync.dma_start(qt_sb[:, i, 1024:2048], qt_r[:, i, 1024:2048])
        nc.sync.dma_start(wq_sb[:], wq.rearrange("(i p) c -> p i c", p=128))
        nc.sync.dma_start(wk_sb[:], wk.rearrange("(i p) c -> p i c", p=128))
        nc.sync.dma_start(wv_sb[:], wv.rearrange("(i p) c -> p i c", p=128))
        nc.sync.dma_start(wo_sb[:], wo.rearrange("(m p) e -> p m e", p=128))
        nc.sync.dma_start(bq_sb[:], bq[:])
        nc.sync.dma_start(bk_sb[:], bk[:])
        nc.sync.dma_start(bv_sb[:], bv[:])
        nc.vector.memset(ones_sb[:], 1.0)
        for si in range(16):
            nc.gpsimd.memset(v_sb[si][:, :, D : D + 1], 1.0)

        # --- qkv projections (q/k chunk 0 first so attention can start early) ---
        with tc.tile_pool(name="pqk", bufs=3, space="PSUM") as pqk, tc.tile_pool(
            name="pv", bufs=3, space="PSUM"
        ) as pv:
            def qk_chunk(m):
                for g in range(4):
                    for wsb, dst, bsb, scale in (
                        (wq_sb, qT_sb, bq_sb, 0.125),
                        (wk_sb, kT_sb, bk_sb, 1.0),
                    ):
                        ps = pqk.tile([128, 512], f32, tag="pqk", name=f"pqk{m}_{g}")
                        for i in range(NE):
                            nc.tensor.matmul(
                                ps[:],
                                lhsT=wsb[:, i, 128 * m : 128 * m + 128],
                                rhs=qt_sb[:, i, 512 * g : 512 * g + 512],
                                start=(i == 0),
                                stop=(i == NE - 1),
                            )
                        if scale == 1.0:
                            nc.vector.tensor_scalar_add(
                                dst[:, m, 512 * g : 512 * g + 512],
                                ps[:],
                                bsb[:, m : m + 1],
                            )
                        else:
                            nc.vector.tensor_scalar(
                                dst[:, m, 512 * g : 512 * g + 512],
                                ps[:],
                                scalar1=bsb[:, m : m + 1],
                                scalar2=scale,
                                op0=mybir.AluOpType.add,
                                op1=mybir.AluOpType.mult,
                            )

            qk_chunk(0)
            for si in range(16):
                ps = pv.tile([128, C], f32, tag="pv", name=f"pv{si}")
                for i in range(NE):
                    nc.tensor.matmul(
                        ps[:],
                        lhsT=qt_sb[:, i, 128 * si : 128 * si + 128],
                        rhs=wv_sb[:, i, :],
                        start=(i == 0),
                        stop=False,
                    )
                nc.tensor.matmul(
                    ps[:],
                    lhsT=ones_sb[:, 0:128],
                    rhs=bv_sb[:],
                    start=False,
                    stop=True,
                )
                nc.vector.tensor_copy(
                    v_sb[si][:, :, 0:D],
                    ps[:].rearrange("p (h d) -> p h d", h=HC),
                )
            qk_chunk(1)

        # --- attention (one head at a time; scores+av psum double-buffered) ---
        with tc.tile_pool(name="psc", bufs=2, space="PSUM") as psc, tc.tile_pool(
            name="pav", bufs=2, space="PSUM"
        ) as pav, tc.tile_pool(name="ppool", bufs=6) as ppool, tc.tile_pool(
            name="rl", bufs=2
        ) as rl:
            for h in range(HC):
                pr, j = h // 2, h % 2
                b0 = 64 * j
                for Hh in range(2):  # q halves of 1024
                    q0 = 1024 * Hh
                    av = pav.tile(
                        [D + 1, 1024], f32, tag="av", name=f"av{h}_{Hh}"
                    )
                    for kc in range(8 * Hh + 8):
                        md = kc - 8 * Hh
                        psj = psc.tile(
                            [128, 1024], f32, tag="sc", name=f"sc{h}_{Hh}_{kc}"
                        )
                        for g2 in range(2):
                            if md >= 4 and g2 == 0:
                                continue
                            nc.tensor.matmul(
                                psj[:, 512 * g2 : 512 * g2 + 512],
                                lhsT=kT_sb[
                                    b0 : b0 + 64, pr, 128 * kc : 128 * kc + 128
                                ],
                                rhs=qT_sb[
                                    b0 : b0 + 64,
                                    pr,
                                    q0 + 512 * g2 : q0 + 512 * g2 + 512,
                                ],
                                start=True,
                                stop=True,
                            )
                        pt = ppool.tile([128, 1024], bf16, tag="p")
                        e0 = max(0, 128 * md)
                        nc.scalar.activation(pt[:, e0:1024], psj[:, e0:1024], Exp)
                        if 0 <= md <= 7:
                            zs = 0 if md < 4 else 512
                            if 128 * md > zs:
                                nc.gpsimd.memset(pt[:, zs : 128 * md], 0.0)
                            blk = pt[:, 128 * md : 128 * md + 128]
                            nc.gpsimd.affine_select(
                                out=blk,
                                in_=blk,
                                pattern=[[1, 128]],
                                compare_op=mybir.AluOpType.is_ge,
                                fill=0.0,
                                base=0,
                                channel_multiplier=-1,
                            )
                        for g2 in range(2):
                            if md >= 4 and g2 == 0:
                                continue
                            nc.tensor.matmul(
                                av[:, 512 * g2 : 512 * g2 + 512],
                                lhsT=v_sb[kc][:, h, :],
                                rhs=pt[:, 512 * g2 : 512 * g2 + 512],
                                start=(kc == 0),
                                stop=(kc == (8 * Hh + 3 if g2 == 0 else 8 * Hh + 7)),
                            )
                    # softmax denom l on psum partition 64 -> 1/l -> broadcast
                    l_sb = rl.tile([D + 1, 1024], f32, tag="l", name=f"l{h}_{Hh}")
                    nc.vector.tensor_copy(l_sb[D : D + 1, :], av[D : D + 1, :])
                    ltall = rl.tile([128, 8], f32, tag="ltall", name=f"lt{h}_{Hh}")
                    l_row = l_sb[D : D + 1, :]
                    nc.sync.dma_start(
                        ltall[:],
                        bass.AP(
                            tensor=l_row.tensor,
                            offset=l_row.offset,
                            ap=[list(l_row.ap[0]), [8, 128], [1, 8]],
                        ),
                    )
                    nc.vector.reciprocal(ltall[:], ltall[:])
                    ridx = 2 * h + Hh
                    nc.sync.dma_start(
                        rrow[ridx, :].rearrange("(p c) -> p c", p=128), ltall[:]
                    )
                    rb = rl.tile([64, 1024], f32, tag="rb", name=f"rb{h}_{Hh}")
                    rr = rrow[ridx, :]
                    nc.sync.dma_start(
                        rb[:],
                        bass.AP(
                            tensor=rr.tensor,
                            offset=rr.offset,
                            ap=[[0, 64], [1, 1024]],
                        ),
                    )
                    nc.vector.tensor_mul(
                        out_sb[b0 : b0 + 64, pr, q0 : q0 + 1024],
                        av[0:D, :],
                        rb[:],
                    )

        # --- output projection (partial: this core's 256 contraction rows) ---
        with tc.tile_pool(name="py", bufs=4, space="PSUM") as py, tc.tile_pool(
            name="ysb", bufs=4
        ) as ysb:
            for t in range(16):
                for e in range(2):
                    ps = py.tile([128, 512], f32, tag="py")
                    for m in range(2):
                        nc.tensor.matmul(
                            ps[:],
                            lhsT=out_sb[:, m, 128 * t : 128 * t + 128],
                            rhs=wo_sb[:, m, 512 * e : 512 * e + 512],
                            start=(m == 0),
                            stop=(m == 1),
                        )
                    yt = ysb.tile([128, 512], f32, tag="yt")
                    if e == 0:
                        nc.vector.tensor_copy(yt[:], ps[:])
                    else:
                        nc.scalar.copy(yt[:], ps[:])
                    nc.sync.dma_start(
                        y[128 * t : 128 * t + 128, 512 * e : 512 * e + 512], yt[:]
                    )


_NC = None


def build_nc():
    global _NC
    if _NC is not None:
        return _NC
    nc = bacc.Bacc("TRN2", target_bir_lowering=False, debug=False, num_devices=8)
    qt = nc.dram_tensor("qt", [E, S], bf16, kind="ExternalInput").ap()
    wq = nc.dram_tensor("wq", [E, C], bf16, kind="ExternalInput").ap()
    wk = nc.dram_tensor("wk", [E, C], bf16, kind="ExternalInput").ap()
    wv = nc.dram_tensor("wv", [E, C], bf16, kind="ExternalInput").ap()
    wo = nc.dram_tensor("wo", [C, E], bf16, kind="ExternalInput").ap()
    bq = nc.dram_tensor("bq", [128, 2], f32, kind="ExternalInput").ap()
    bk = nc.dram_tensor("bk", [128, 2], f32, kind="ExternalInput").ap()
    bv = nc.dram_tensor("bv", [1, C], bf16, kind="ExternalInput").ap()
    y = nc.dram_tensor("y", [S, E], f32, kind="ExternalOutput").ap()
    with tile.TileContext(nc) as tc:
        _build_kernel(tc, qt, wq, wk, wv, wo, bq, bk, bv, y)
    nc.compile()
    _NC = nc
    return nc


def make_in_maps(Q, Wqkv, bqkv):
    """Per-core input dicts (8 cores: batch-major, then head-group)."""
    in_maps = []
    for c in range(8):
        b, hq = c // 4, c % 4
        cs = C * hq
        qt_np = np.ascontiguousarray(Q[b].T).astype(bfnp)
        wq_np = np.ascontiguousarray(Wqkv[:, cs : cs + C]).astype(bfnp)
        wk_np = np.ascontiguousarray(Wqkv[:, E + cs : E + cs + C]).astype(bfnp)
        wv_np = np.ascontiguousarray(Wqkv[:, 2 * E + cs : 2 * E + cs + C]).astype(bfnp)
        bq_np = np.ascontiguousarray(
            bqkv[cs : cs + C].astype(np.float32).reshape(2, 128).T
        )
        bk_np = np.ascontiguousarray(
            bqkv[E + cs : E + cs + C].astype(np.float32).reshape(2, 128).T
        )
        bv_np = bqkv[2 * E + cs : 2 * E + cs + C].reshape(1, C).astype(bfnp)
        in_maps.append(
            {
                "qt": qt_np,
                "wq": wq_np,
                "wk": wk_np,
                "wv": wv_np,
                "bq": bq_np,
                "bk": bk_np,
                "bv": bv_np,
            }
        )
    return in_maps


def kernel(Q, Wqkv, bqkv, Wout, bout, _trace=False, _trace_kwargs=None):
    Q = np.asarray(Q, dtype=np.float32)
    Wqkv = np.asarray(Wqkv, dtype=np.float32)
    bqkv = np.asarray(bqkv, dtype=np.float32)
    Wout = np.asarray(Wout, dtype=np.float32)
    bout = np.asarray(bout, dtype=np.float32)

    nc = build_nc()
    in_maps = make_in_maps(Q, Wqkv, bqkv)
    for c in range(8):
        hq = c % 4
        cs = C * hq
        in_maps[c]["wo"] = np.ascontiguousarray(Wout[cs : cs + C, :]).astype(bfnp)

    kwargs = {}
    if _trace:
        kwargs = dict(trace=True, trace_cores=list(range(8)))
        if _trace_kwargs:
            kwargs.update(_trace_kwargs)
    res = run_bass_kernel_spmd(nc, in_maps, core_ids=list(range(8)), **kwargs)

    out = np.zeros((2, S, E), dtype=np.float32)
    for c in range(8):
        out[c // 4] += np.asarray(res.results[c]["y"], dtype=np.float32)
    out += bout.astype(np.float32)[None, None, :]
    if _trace:
        kernel._last_results = res
    return out
